# revision 8
# baseline (speedup 1.0000x reference)
"""Trainium2 Bass kernel for nn_NNFFTLayer (radix-R butterfly mix layer).

Reference computation (per position p, last dim N=8192):
    scale = tile(weights, R)                  # weights: [1024], R=8 -> [8192]
    y     = (scale * x).reshape(..., 64, 8, 16)   # [k, i, c]
    out[..., k, j, c] = sum_i lin_weights[j, i] * y[..., k, i, c]

Each 128-element chunk k of the last dim undergoes an independent linear map
M_km (km = k % 8) that folds the scale and the 8x8 mix:
    M_km[j*16+c', i*16+c] = L[j,i] * weights[km*128 + i*16 + c] * (c' == c)

Device strategy (feature-sharded over 8 cores, 8 chunks each):
  - host casts x to bf16 and transposes to X^T [8192 feat, 8192 pos]:
    HBM-bandwidth bound, so bf16 halves the bytes (~0.3% rel err, gate
    2e-2); the transposed feature-major layout eliminates all on-chip
    transposes AND gives maximal 16 KiB contiguous DMA descriptor lines
    (sustained HBM rate was measured to degrade with small descriptors).
  - core c handles feature rows [c*1024, (c+1)*1024): 8 slabs of one
    128-row chunk x 8192 positions; chunk km = slab index for every core,
    so each slab is 16 matmuls vs one resident stationary M_km^T
    (rhs = X^T slab, 512 positions per matmul) -> f32 PSUM,
    DVE/ACT copies (alternating) downcast PSUM -> bf16 out slab
  - DMA 2 MiB slabs in/out; host reassembles Y^T, transposes, upcasts.
  ~16 MiB in + 16 MiB out per core.
"""

import sys

if "/opt/trn_rl_repo" not in sys.path:
    sys.path.insert(0, "/opt/trn_rl_repo")

import numpy as np
import ml_dtypes

BF16 = ml_dtypes.bfloat16

P = 128
N = 8192
R = 8
TWO_R = 16
N_CHUNKS = N // P        # 64 feature chunks
KM = 1024 // P           # 8 distinct per-chunk matrices
N_CORES = 8
POS_TOTAL = 4 * 2048     # 8192 positions (batch*seq)
ROWS_PER_CORE = N // N_CORES          # 1024 feature rows per core
SLABS = ROWS_PER_CORE // P            # 8 slabs (= chunks) per core
HB = 512                              # matmul free size (1 PSUM bank f32)
NH = POS_TOTAL // HB                  # 16 h-blocks per slab

_CACHE = {}


def _build_nc():
    import concourse.bacc as bacc
    import concourse.mybir as mybir
    import concourse.tile as tile

    nc = bacc.Bacc("TRN2", target_bir_lowering=False, debug=False)
    f32 = mybir.dt.float32
    bf16 = mybir.dt.bfloat16
    # xs/out hold this core's rows of X^T / Y^T: [feature row, position]
    xs = nc.dram_tensor("xs", (ROWS_PER_CORE, POS_TOTAL), bf16, kind="ExternalInput")
    mt = nc.dram_tensor("mt", (P, KM * P), bf16, kind="ExternalInput")
    out = nc.dram_tensor("out", (ROWS_PER_CORE, POS_TOTAL), bf16, kind="ExternalOutput")

    CW = 2 * HB              # 1024: copy unit (2 PSUM banks per mm tile)

    with tile.TileContext(nc) as tc:
        with (
            tc.tile_pool(name="singles", bufs=1) as singles,
            tc.tile_pool(name="xin", bufs=3) as xin,
            tc.tile_pool(name="outp", bufs=2) as outp,
            tc.tile_pool(name="mm_ps", bufs=4, space="PSUM") as mm_ps,
        ):
            # mt rides the ACT ring so the first x slab is first on SP
            mt_sb = singles.tile([P, KM * P], bf16)
            nc.scalar.dma_start(mt_sb[:], mt[:, :])

            # 8 slabs of 2 MiB: 16 KiB descriptor lines for peak HBM rate,
            # and 16 back-to-back matmuls per slab keep the PE's p-state
            # ramp warm (idle gaps drop the PE clock 2.4 -> 1.2/0.65 GHz)
            cu = 0
            for s in range(SLABS):
                xsb = xin.tile([P, POS_TOTAL], bf16)
                pieces = 2 if s == 0 else 1
                pw = POS_TOTAL // pieces
                for p in range(pieces):
                    nc.sync.dma_start(
                        xsb[:, p * pw:(p + 1) * pw],
                        xs[s * P:(s + 1) * P, p * pw:(p + 1) * pw],
                    )
                osb = outp.tile([P, POS_TOTAL], bf16)
                for t in range(POS_TOTAL // CW):
                    mm = mm_ps.tile([P, CW], f32)
                    for h in range(2):
                        nc.tensor.matmul(
                            mm[:, h * HB:(h + 1) * HB],
                            lhsT=mt_sb[:, s * P:(s + 1) * P],
                            rhs=xsb[:, t * CW + h * HB:t * CW + (h + 1) * HB],
                            start=True, stop=True,
                        )
                    # one 1024-wide copy per 2-bank tile halves the per-copy
                    # fixed overhead; alternate DVE/ACT to split the load
                    ceng = nc.vector.tensor_copy if cu % 2 == 0 else nc.scalar.copy
                    cu += 1
                    ceng(osb[:, t * CW:(t + 1) * CW], mm[:])
                # drain phase: split the last two slabs' stores across both
                # rings — a single queue under the HW duty-cycle throttle
                # runs at half rate, two queues together still hit peak
                if s >= SLABS - 2:
                    for v in range(2):
                        seng = nc.sync if v == 0 else nc.scalar
                        hp = POS_TOTAL // 2
                        seng.dma_start(
                            out[s * P:(s + 1) * P, v * hp:(v + 1) * hp],
                            osb[:, v * hp:(v + 1) * hp],
                        )
                else:
                    nc.scalar.dma_start(out[s * P:(s + 1) * P, :], osb[:])

    # Strip the framework's const-register memsets and the entry all-engine
    # barrier: the memsets' GpSimd library load (~6us Q7 boot) gates the
    # barrier and delays kernel start, and with them gone the barrier
    # protects nothing — register init is per-engine (engines are in-order)
    # and the tile context's own semaphores carry all cross-engine deps.
    entry = nc.main_func.blocks[0]
    entry.instructions = [
        i for i in entry.instructions
        if not isinstance(i, (mybir.InstMemset, mybir.InstDrain,
                              mybir.InstEventSemaphore))
    ]

    nc.compile()
    return nc


def _get_nc():
    if "nc" not in _CACHE:
        _CACHE["nc"] = _build_nc()
    return _CACHE["nc"]


def build_mt(weights, lin_weights):
    """[P, KM*P] table; column block km holds M_km^T (matmul lhsT layout)."""
    L = np.asarray(lin_weights, np.float32)
    w = np.asarray(weights, np.float32)
    a = np.arange(P)   # out index within chunk: a = j*16 + c'
    b = np.arange(P)   # in  index within chunk: b = i*16 + c
    mix = L[a[:, None] // TWO_R, b[None, :] // TWO_R] * (
        (a[:, None] % TWO_R) == (b[None, :] % TWO_R)
    ).astype(np.float32)
    mt = np.zeros((P, KM * P), np.float32)
    for km in range(KM):
        M = mix * w[km * P + b][None, :]       # [a, b]
        mt[:, km * P:(km + 1) * P] = M.T       # lhsT[b, a] = M[a, b]
    return np.ascontiguousarray(mt)


def prep_in_maps(x, weights, lin_weights):
    xflat = np.asarray(x, np.float32).reshape(POS_TOTAL, N).astype(BF16)
    xT = np.ascontiguousarray(xflat.T)         # [N feat, POS_TOTAL]
    mt_host = build_mt(weights, lin_weights).astype(BF16)
    return [
        {"xs": xT[c * ROWS_PER_CORE:(c + 1) * ROWS_PER_CORE],
         "mt": mt_host}
        for c in range(N_CORES)
    ]


def unpack_out(res, shape):
    yT = np.concatenate(
        [res.results[c]["out"] for c in range(N_CORES)], axis=0
    )                                          # [N feat, POS_TOTAL] bf16
    return yT.T.astype(np.float32).reshape(shape)


def kernel(x, weights, lin_weights):
    from concourse import bass_utils

    nc = _get_nc()
    in_maps = prep_in_maps(x, weights, lin_weights)
    res = bass_utils.run_bass_kernel_spmd(nc, in_maps, core_ids=list(range(N_CORES)))
    return unpack_out(res, np.asarray(x).shape)


# revision 11
# speedup vs baseline: 1.0280x; 1.0280x over previous
"""Trainium2 Bass kernel for nn_NNFFTLayer (radix-R butterfly mix layer).

Reference computation (per position p, last dim N=8192):
    scale = tile(weights, R)                  # weights: [1024], R=8 -> [8192]
    y     = (scale * x).reshape(..., 64, 8, 16)   # [k, i, c]
    out[..., k, j, c] = sum_i lin_weights[j, i] * y[..., k, i, c]

Each 128-element chunk k of the last dim undergoes an independent linear map
M_km (km = k % 8) that folds the scale and the 8x8 mix:
    M_km[j*16+c', i*16+c] = L[j,i] * weights[km*128 + i*16 + c] * (c' == c)

Device strategy (feature-sharded over 8 cores, 8 chunks each):
  - host casts x to bf16 and transposes to X^T [8192 feat, 8192 pos]:
    HBM-bandwidth bound, so bf16 halves the bytes (~0.3% rel err, gate
    2e-2); the transposed feature-major layout eliminates all on-chip
    transposes AND gives maximal 16 KiB contiguous DMA descriptor lines
    (sustained HBM rate was measured to degrade with small descriptors).
  - core c handles feature rows [c*1024, (c+1)*1024): 8 slabs of one
    128-row chunk x 8192 positions; chunk km = slab index for every core,
    so each slab is 16 matmuls vs one resident stationary M_km^T
    (rhs = X^T slab, 512 positions per matmul) -> f32 PSUM,
    DVE/ACT copies (alternating) downcast PSUM -> bf16 out slab
  - DMA 2 MiB slabs in/out; host reassembles Y^T, transposes, upcasts.
  ~16 MiB in + 16 MiB out per core.
"""

import sys

if "/opt/trn_rl_repo" not in sys.path:
    sys.path.insert(0, "/opt/trn_rl_repo")

import numpy as np
import ml_dtypes

BF16 = ml_dtypes.bfloat16

P = 128
N = 8192
R = 8
TWO_R = 16
N_CHUNKS = N // P        # 64 feature chunks
KM = 1024 // P           # 8 distinct per-chunk matrices
N_CORES = 8
POS_TOTAL = 4 * 2048     # 8192 positions (batch*seq)
ROWS_PER_CORE = N // N_CORES          # 1024 feature rows per core
SLABS = ROWS_PER_CORE // P            # 8 slabs (= chunks) per core
HB = 512                              # matmul free size (1 PSUM bank f32)
NH = POS_TOTAL // HB                  # 16 h-blocks per slab

_CACHE = {}


def _build_nc():
    import concourse.bacc as bacc
    import concourse.mybir as mybir
    import concourse.tile as tile

    nc = bacc.Bacc("TRN2", target_bir_lowering=False, debug=False)
    f32 = mybir.dt.float32
    bf16 = mybir.dt.bfloat16
    # xs/out hold this core's rows of X^T / Y^T: [feature row, position]
    xs = nc.dram_tensor("xs", (ROWS_PER_CORE, POS_TOTAL), bf16, kind="ExternalInput")
    mt = nc.dram_tensor("mt", (P, KM * P), bf16, kind="ExternalInput")
    out = nc.dram_tensor("out", (ROWS_PER_CORE, POS_TOTAL), bf16, kind="ExternalOutput")

    CW = 2 * HB              # 1024: copy unit (2 PSUM banks per mm tile)

    with tile.TileContext(nc) as tc:
        with (
            tc.tile_pool(name="singles", bufs=1) as singles,
            tc.tile_pool(name="xin", bufs=3) as xin,
            tc.tile_pool(name="outp", bufs=2) as outp,
            tc.tile_pool(name="mm_ps", bufs=4, space="PSUM") as mm_ps,
        ):
            # mt rides the ACT ring so the first x slab is first on SP
            mt_sb = singles.tile([P, KM * P], bf16)
            nc.scalar.dma_start(mt_sb[:], mt[:, :])

            # 8 slabs of 2 MiB: 16 KiB descriptor lines for peak HBM rate,
            # and 16 back-to-back matmuls per slab keep the PE's p-state
            # ramp warm (idle gaps drop the PE clock 2.4 -> 1.2/0.65 GHz)
            cu = 0
            for s in range(SLABS):
                xsb = xin.tile([P, POS_TOTAL], bf16)
                pieces = 2 if s == 0 else 1
                pw = POS_TOTAL // pieces
                for p in range(pieces):
                    nc.sync.dma_start(
                        xsb[:, p * pw:(p + 1) * pw],
                        xs[s * P:(s + 1) * P, p * pw:(p + 1) * pw],
                    )
                osb = outp.tile([P, POS_TOTAL], bf16)
                for t in range(POS_TOTAL // CW):
                    mm = mm_ps.tile([P, CW], f32)
                    for h in range(2):   # HW caps matmul free size at 512
                        nc.tensor.matmul(
                            mm[:, h * HB:(h + 1) * HB],
                            lhsT=mt_sb[:, s * P:(s + 1) * P],
                            rhs=xsb[:, t * CW + h * HB:t * CW + (h + 1) * HB],
                            start=True, stop=True,
                        )
                    # one 1024-wide copy per tile halves the per-copy fixed
                    # overhead; alternate DVE/ACT to split the load
                    ceng = nc.vector.tensor_copy if cu % 2 == 0 else nc.scalar.copy
                    cu += 1
                    ceng(osb[:, t * CW:(t + 1) * CW], mm[:])
                # drain phase: split the last two slabs' stores across both
                # rings — a single queue under the HW duty-cycle throttle
                # runs at half rate, two queues together still hit peak
                if s >= SLABS - 2:
                    for v in range(2):
                        seng = nc.sync if v == 0 else nc.scalar
                        hp = POS_TOTAL // 2
                        seng.dma_start(
                            out[s * P:(s + 1) * P, v * hp:(v + 1) * hp],
                            osb[:, v * hp:(v + 1) * hp],
                        )
                else:
                    nc.scalar.dma_start(out[s * P:(s + 1) * P, :], osb[:])

    # Strip the framework's const-register memsets and the entry all-engine
    # barrier: the memsets' GpSimd library load (~6us Q7 boot) gates the
    # barrier and delays kernel start, and with them gone the barrier
    # protects nothing — register init is per-engine (engines are in-order)
    # and the tile context's own semaphores carry all cross-engine deps.
    entry = nc.main_func.blocks[0]
    entry.instructions = [
        i for i in entry.instructions
        if not isinstance(i, (mybir.InstMemset, mybir.InstDrain,
                              mybir.InstEventSemaphore))
    ]

    nc.compile()

    # Drop redundant PE weight reloads: every matmul in a slab uses the same
    # stationary matrix, and compile() splits each into Ldweights+Matmult.
    # A duplicate Ldweights (same weights AP) with no waits/updates is a pure
    # ~126 ns PE stall; the weights are already resident in the array.
    for b in nc.main_func.blocks:
        kept, prev_sig = [], None
        for i in b.instructions:
            if isinstance(i, mybir.InstLdweights):
                sig = str(i.ins[0])
                if sig == prev_sig and not i.has_wait() and not i.has_update():
                    continue
                prev_sig = sig
            kept.append(i)
        b.instructions = kept
    return nc


def _get_nc():
    if "nc" not in _CACHE:
        _CACHE["nc"] = _build_nc()
    return _CACHE["nc"]


def build_mt(weights, lin_weights):
    """[P, KM*P] table; column block km holds M_km^T (matmul lhsT layout)."""
    L = np.asarray(lin_weights, np.float32)
    w = np.asarray(weights, np.float32)
    a = np.arange(P)   # out index within chunk: a = j*16 + c'
    b = np.arange(P)   # in  index within chunk: b = i*16 + c
    mix = L[a[:, None] // TWO_R, b[None, :] // TWO_R] * (
        (a[:, None] % TWO_R) == (b[None, :] % TWO_R)
    ).astype(np.float32)
    mt = np.zeros((P, KM * P), np.float32)
    for km in range(KM):
        M = mix * w[km * P + b][None, :]       # [a, b]
        mt[:, km * P:(km + 1) * P] = M.T       # lhsT[b, a] = M[a, b]
    return np.ascontiguousarray(mt)


def prep_in_maps(x, weights, lin_weights):
    xflat = np.asarray(x, np.float32).reshape(POS_TOTAL, N).astype(BF16)
    xT = np.ascontiguousarray(xflat.T)         # [N feat, POS_TOTAL]
    mt_host = build_mt(weights, lin_weights).astype(BF16)
    return [
        {"xs": xT[c * ROWS_PER_CORE:(c + 1) * ROWS_PER_CORE],
         "mt": mt_host}
        for c in range(N_CORES)
    ]


def unpack_out(res, shape):
    yT = np.concatenate(
        [res.results[c]["out"] for c in range(N_CORES)], axis=0
    )                                          # [N feat, POS_TOTAL] bf16
    return yT.T.astype(np.float32).reshape(shape)


def kernel(x, weights, lin_weights):
    from concourse import bass_utils

    nc = _get_nc()
    in_maps = prep_in_maps(x, weights, lin_weights)
    res = bass_utils.run_bass_kernel_spmd(nc, in_maps, core_ids=list(range(N_CORES)))
    return unpack_out(res, np.asarray(x).shape)


# revision 14
# speedup vs baseline: 1.1375x; 1.1065x over previous
"""Trainium2 Bass kernel for nn_NNFFTLayer (radix-R butterfly mix layer).

Reference computation (per position p, last dim N=8192):
    scale = tile(weights, R)                  # weights: [1024], R=8 -> [8192]
    y     = (scale * x).reshape(..., 64, 8, 16)   # [k, i, c]
    out[..., k, j, c] = sum_i lin_weights[j, i] * y[..., k, i, c]

Each 128-element chunk k of the last dim undergoes an independent linear map
M_km (km = k % 8) that folds the scale and the 8x8 mix:
    M_km[j*16+c', i*16+c] = L[j,i] * weights[km*128 + i*16 + c] * (c' == c)

Device strategy (feature-sharded over 8 cores, 8 chunks each):
  - host casts x to bf16 and transposes to X^T [8192 feat, 8192 pos]:
    HBM-bandwidth bound, so bf16 halves the bytes (~0.3% rel err, gate
    2e-2); the transposed feature-major layout eliminates all on-chip
    transposes AND gives maximal 16 KiB contiguous DMA descriptor lines
    (sustained HBM rate was measured to degrade with small descriptors).
  - core c handles feature rows [c*1024, (c+1)*1024): 8 slabs of one
    128-row chunk x 8192 positions; chunk km = slab index for every core,
    so each slab is 16 matmuls vs one resident stationary M_km^T
    (rhs = X^T slab, 512 positions per matmul) -> f32 PSUM,
    DVE/ACT copies (alternating) downcast PSUM -> bf16 out slab
  - DMA 2 MiB slabs in/out; host reassembles Y^T, transposes, upcasts.
  ~16 MiB in + 16 MiB out per core.
"""

import sys

if "/opt/trn_rl_repo" not in sys.path:
    sys.path.insert(0, "/opt/trn_rl_repo")

import numpy as np
import ml_dtypes

BF16 = ml_dtypes.bfloat16

P = 128
N = 8192
R = 8
TWO_R = 16
N_CHUNKS = N // P        # 64 feature chunks
KM = 1024 // P           # 8 distinct per-chunk matrices
N_CORES = 8
POS_TOTAL = 4 * 2048     # 8192 positions (batch*seq)
ROWS_PER_CORE = N // N_CORES          # 1024 feature rows per core
SLABS = ROWS_PER_CORE // P            # 8 slabs (= chunks) per core
HB = 512                              # matmul free size (1 PSUM bank f32)
NH = POS_TOTAL // HB                  # 16 h-blocks per slab

_CACHE = {}


def _build_nc():
    import concourse.bacc as bacc
    import concourse.mybir as mybir
    import concourse.tile as tile

    nc = bacc.Bacc("TRN2", target_bir_lowering=False, debug=False)
    f32 = mybir.dt.float32
    bf16 = mybir.dt.bfloat16
    # xs/out hold this core's rows of X^T / Y^T: [feature row, position]
    xs = nc.dram_tensor("xs", (ROWS_PER_CORE, POS_TOTAL), bf16, kind="ExternalInput")
    mt = nc.dram_tensor("mt", (P, KM * P), bf16, kind="ExternalInput")
    out = nc.dram_tensor("out", (ROWS_PER_CORE, POS_TOTAL), bf16, kind="ExternalOutput")

    CW = 2 * HB              # 1024: copy unit (2 PSUM banks per mm tile)

    with tile.TileContext(nc) as tc:
        with (
            tc.tile_pool(name="singles", bufs=1) as singles,
            tc.tile_pool(name="xin", bufs=4) as xin,
            tc.tile_pool(name="outp", bufs=3) as outp,
            tc.tile_pool(name="mm_ps", bufs=4, space="PSUM") as mm_ps,
        ):
            # mt rides the ACT ring so the first x slab is first on SP
            mt_sb = singles.tile([P, KM * P], bf16)
            nc.scalar.dma_start(mt_sb[:], mt[:, :])

            # 8 slabs of 2 MiB: 16 KiB descriptor lines for peak HBM rate,
            # and 16 back-to-back matmuls per slab keep the PE's p-state
            # ramp warm (idle gaps drop the PE clock 2.4 -> 1.2/0.65 GHz)
            cu = 0
            for s in range(SLABS):
                xsb = xin.tile([P, POS_TOTAL], bf16)
                # two 1 MiB pieces per slab (8 KiB lines sustain the same
                # HBM rate as 16 KiB) so compute can start on the first half
                # while the second streams — smooths PE's arrival cadence
                pw = POS_TOTAL // 2
                for p in range(2):
                    nc.sync.dma_start(
                        xsb[:, p * pw:(p + 1) * pw],
                        xs[s * P:(s + 1) * P, p * pw:(p + 1) * pw],
                    )
                osb = outp.tile([P, POS_TOTAL], bf16)
                for t in range(POS_TOTAL // CW):
                    mm = mm_ps.tile([P, CW], f32)
                    for h in range(2):   # HW caps matmul free size at 512
                        nc.tensor.matmul(
                            mm[:, h * HB:(h + 1) * HB],
                            lhsT=mt_sb[:, s * P:(s + 1) * P],
                            rhs=xsb[:, t * CW + h * HB:t * CW + (h + 1) * HB],
                            start=True, stop=True,
                        )
                    # one 1024-wide copy per tile halves the per-copy fixed
                    # overhead; alternate DVE/ACT to split the load
                    ceng = nc.vector.tensor_copy if cu % 2 == 0 else nc.scalar.copy
                    cu += 1
                    ceng(osb[:, t * CW:(t + 1) * CW], mm[:])
                # stores in 1 MiB halves so each starts as soon as its
                # copies land; drain phase (last two slabs) splits across
                # both rings — a single queue under the HW duty-cycle
                # throttle runs at half rate, two together still hit peak
                for v in range(2):
                    seng = nc.sync if (s >= SLABS - 2 and v == 0) else nc.scalar
                    seng.dma_start(
                        out[s * P:(s + 1) * P, v * pw:(v + 1) * pw],
                        osb[:, v * pw:(v + 1) * pw],
                    )

    # Strip the framework's const-register memsets and the entry all-engine
    # barrier: the memsets' GpSimd library load (~6us Q7 boot) gates the
    # barrier and delays kernel start, and with them gone the barrier
    # protects nothing — register init is per-engine (engines are in-order)
    # and the tile context's own semaphores carry all cross-engine deps.
    entry = nc.main_func.blocks[0]
    entry.instructions = [
        i for i in entry.instructions
        if not isinstance(i, (mybir.InstMemset, mybir.InstDrain,
                              mybir.InstEventSemaphore))
    ]

    nc.compile()

    # Drop redundant PE weight reloads: every matmul in a slab uses the same
    # stationary matrix, and compile() splits each into Ldweights+Matmult.
    # A duplicate Ldweights (same weights AP) with no waits/updates is a pure
    # ~126 ns PE stall; the weights are already resident in the array.
    for b in nc.main_func.blocks:
        kept, prev_sig = [], None
        for i in b.instructions:
            if isinstance(i, mybir.InstLdweights):
                sig = str(i.ins[0])
                if sig == prev_sig and not i.has_wait() and not i.has_update():
                    continue
                prev_sig = sig
            kept.append(i)
        b.instructions = kept
    return nc


def _get_nc():
    if "nc" not in _CACHE:
        _CACHE["nc"] = _build_nc()
    return _CACHE["nc"]


def build_mt(weights, lin_weights):
    """[P, KM*P] table; column block km holds M_km^T (matmul lhsT layout)."""
    L = np.asarray(lin_weights, np.float32)
    w = np.asarray(weights, np.float32)
    a = np.arange(P)   # out index within chunk: a = j*16 + c'
    b = np.arange(P)   # in  index within chunk: b = i*16 + c
    mix = L[a[:, None] // TWO_R, b[None, :] // TWO_R] * (
        (a[:, None] % TWO_R) == (b[None, :] % TWO_R)
    ).astype(np.float32)
    mt = np.zeros((P, KM * P), np.float32)
    for km in range(KM):
        M = mix * w[km * P + b][None, :]       # [a, b]
        mt[:, km * P:(km + 1) * P] = M.T       # lhsT[b, a] = M[a, b]
    return np.ascontiguousarray(mt)


def prep_in_maps(x, weights, lin_weights):
    xflat = np.asarray(x, np.float32).reshape(POS_TOTAL, N).astype(BF16)
    xT = np.ascontiguousarray(xflat.T)         # [N feat, POS_TOTAL]
    mt_host = build_mt(weights, lin_weights).astype(BF16)
    return [
        {"xs": xT[c * ROWS_PER_CORE:(c + 1) * ROWS_PER_CORE],
         "mt": mt_host}
        for c in range(N_CORES)
    ]


def unpack_out(res, shape):
    yT = np.concatenate(
        [res.results[c]["out"] for c in range(N_CORES)], axis=0
    )                                          # [N feat, POS_TOTAL] bf16
    return yT.T.astype(np.float32).reshape(shape)


def kernel(x, weights, lin_weights):
    from concourse import bass_utils

    nc = _get_nc()
    in_maps = prep_in_maps(x, weights, lin_weights)
    res = bass_utils.run_bass_kernel_spmd(nc, in_maps, core_ids=list(range(N_CORES)))
    return unpack_out(res, np.asarray(x).shape)
